# revision 7
# baseline (speedup 1.0000x reference)
"""Causal self-attention (T=2048, D=2048, H=16) on 8 Trainium2 NeuronCores.

Head-sharded tensor parallel: 2 heads per core. Each core computes its heads'
q/k/v projections (bf16), causal attention, then projects its OWN 256
attention-output features through the matching W_proj columns (input read
straight from SBUF — no collective before the projection) producing a partial
y^T [D, T]; a per-tq-block ReduceScatter sums partials across cores and hands
core r rows [256r, 256r+256) — exactly its output slice. The host
concatenates and casts bf16 -> fp32.

Layouts (all feature/d-major so the PE contracts along partitions):
  - xT      [D, T]   bf16 : x transposed (host-side)
  - wqkvT   [D, 768] bf16 : this core's W_attn rows (q0 q1 k0 k1 v0 v1), transposed
  - wpL     [256, D] bf16 : W_proj[:, own 256 cols].T (din-major)
  - qkT     [128, 4T] bf16: q/k per head, feature on partition
  - v_sb    [tok, 256] bf16 (token on partition) so P.T@V needs no transpose
  - S_T     [tk, tq] so softmax sums run via an all-ones matmul on the PE
Attention is one flat software-pipelined stream over (j, head, tk): the S
matmul + exp for step i+LOOKAHEAD are emitted before the sum/PV matmuls of
step i, so the PE never waits on the ACT exp round trip.  The projection for
block j is emitted right after its normalize and its ReduceScatter overlaps
attention of block j+1.
"""

import numpy as np
import ml_dtypes

import concourse.bacc as bacc
import concourse.bass_utils as bass_utils
import concourse.mybir as mybir
import concourse.tile as tile

T = 2048
D = 2048
H = 16
C = 128
N_CORES = 8
HPC = H // N_CORES          # heads per core = 2
FPC = HPC * C               # features per core = 256
TQB = 512                   # tq block (PSUM free-dim limit for fp32)
NTQ = T // TQB              # 4
NKT = T // 128              # 16 tk tiles
ND = D // 128               # 16 contraction tiles
LOOKAHEAD = 3
SCALE = 1.0 / np.sqrt(np.float32(C))

BF = mybir.dt.bfloat16
F32 = mybir.dt.float32

_NC_CACHE = {}


def build_nc(sim_single_core=False, reps=1):
    key = ("sim" if sim_single_core else "nc") + f"_{reps}"
    if key in _NC_CACHE:
        return _NC_CACHE[key]
    ndev = 1 if sim_single_core else N_CORES
    nc = bacc.Bacc("TRN2", target_bir_lowering=False, debug=False, num_devices=ndev)

    xT = nc.dram_tensor("xT", [D, T], BF, kind="ExternalInput").ap()
    wqkvT = nc.dram_tensor("wqkvT", [D, 3 * FPC], BF, kind="ExternalInput").ap()
    wpL = nc.dram_tensor("wpL", [FPC, D], BF, kind="ExternalInput").ap()
    # mask band: maskB[p, j] = 1.0 if p <= j - 384 else 0.0  (j in [0, 896))
    maskB = nc.dram_tensor("maskB", [128, 896], BF, kind="ExternalInput").ap()
    yb = [nc.dram_tensor(f"yb{j}", [FPC, TQB], BF, kind="ExternalOutput").ap()
          for j in range(NTQ)]
    y_part = [nc.dram_tensor(f"yp{j}", [D, TQB], BF, kind="Internal").ap()
              for j in range(NTQ)]
    rs_out = [nc.dram_tensor(f"rs{j}", [FPC, TQB], BF, kind="Internal").ap()
              for j in range(NTQ)]

    with tile.TileContext(nc) as tc:
        with tc.tile_pool(name="persist", bufs=1) as pp, \
             tc.tile_pool(name="ptiles", bufs=8) as ppt, \
             tc.tile_pool(name="small", bufs=2) as smp, \
             tc.tile_pool(name="osb", bufs=4) as osp, \
             tc.tile_pool(name="yout", bufs=4) as yp, \
             tc.tile_pool(name="psA", bufs=4, space="PSUM") as psA, \
             tc.tile_pool(name="psB", bufs=2, space="PSUM") as psB, \
             tc.tile_pool(name="psC", bufs=2, space="PSUM") as psC:

            mask_sb = pp.tile([128, 896], BF, tag="mask")
            ones_f = pp.tile([128, 128], F32, tag="onesf")
            ones_b = pp.tile([128, 128], BF, tag="onesb")
            nc.vector.memset(ones_f[:], 1.0)
            nc.vector.tensor_copy(ones_b[:], ones_f[:])
            nc.sync.dma_start(mask_sb[:], maskB[:])

            for _rep in range(reps):
                emit_body(nc, tc, pp, ppt, smp, osp, yp, psA, psB, psC,
                          xT, wqkvT, wpL, yb, y_part, rs_out,
                          mask_sb, ones_b, sim_single_core)

    nc.compile()
    _NC_CACHE[key] = nc
    return nc


def emit_body(nc, tc, pp, ppt, smp, osp, yp, psA, psB, psC,
              xT, wqkvT, wpL, yb, y_part, rs_out, mask_sb, ones_b,
              sim_single_core):
    # ---- phase 1: QKV projections ----
    # qkT layout: feature-block fb in {q_h0, q_h1, k_h0, k_h1} at cols
    # [fb*T, (fb+1)*T); v_sb: tok-tile tt at cols [tt*FPC, ...).
    qkT = pp.tile([128, 4 * T], BF, tag="qkT")               # 16KB/part
    v_sb = pp.tile([128, NKT * FPC], BF, tag="v")            # 8KB/part
    ph1_cm = tc.tile_pool(name="ph1", bufs=1)
    sp_cm = tc.tile_pool(name="stream", bufs=2)
    ph1 = ph1_cm.__enter__()
    sp = sp_cm.__enter__()
    w_sb = ph1.tile([128, ND * 3 * FPC], BF, tag="wbig")     # 24KB/part
    wpL_sb = pp.tile([128, HPC * D], BF, tag="wp")           # 8KB/part
    xcols = []
    for tb in range(NTQ):
        xcols.append(sp.tile([128, ND * TQB], BF, tag="xcol", name=f"xcol{tb}"))
    # interleave DMA emission so the first-needed tiles land first
    for t in range(ND):
        nc.sync.dma_start(
            w_sb[:, t * 3 * FPC:(t + 1) * 3 * FPC],
            wqkvT[t * 128:(t + 1) * 128, :])
        nc.sync.dma_start(
            xcols[0][:, t * TQB:(t + 1) * TQB],
            xT[t * 128:(t + 1) * 128, 0:TQB])
    for h in range(HPC):
        nc.sync.dma_start(
            wpL_sb[:, h * D:(h + 1) * D], wpL[h * 128:(h + 1) * 128, :])
    for tb in range(NTQ):
        xcol = xcols[tb]
        if tb > 0:
            for t in range(ND):
                nc.sync.dma_start(
                    xcol[:, t * TQB:(t + 1) * TQB],
                    xT[t * 128:(t + 1) * 128, tb * TQB:(tb + 1) * TQB])
        for fb in range(4):
            ps = psA.tile([128, TQB], F32, tag="a")
            for t in range(ND):
                nc.tensor.matmul(
                    ps[:],
                    w_sb[:, t * 3 * FPC + fb * 128: t * 3 * FPC + fb * 128 + 128],
                    xcol[:, t * TQB:(t + 1) * TQB],
                    start=(t == 0), stop=(t == ND - 1))
            nc.vector.tensor_copy(
                qkT[:, fb * T + tb * TQB: fb * T + (tb + 1) * TQB], ps[:])
        for tt in range(4):
            tok = tb * 4 + tt
            ps = psB.tile([128, FPC], F32, tag="b")
            for t in range(ND):
                nc.tensor.matmul(
                    ps[:],
                    xcol[:, t * TQB + tt * 128: t * TQB + (tt + 1) * 128],
                    w_sb[:, t * 3 * FPC + 2 * FPC:(t + 1) * 3 * FPC],
                    start=(t == 0), stop=(t == ND - 1))
            nc.vector.tensor_copy(v_sb[:, tok * FPC:(tok + 1) * FPC], ps[:])

    sp_cm.__exit__(None, None, None)
    ph1_cm.__exit__(None, None, None)

    # ---- phase 2+3: flat attention + projection stream ----
    steps = [(j, h, tk)
             for j in range(NTQ) for h in range(HPC) for tk in range(4 * (j + 1))]
    n_steps = len(steps)

    p_tiles = {}

    def emit_s_exp(idx):
        # Trapezoid: for a diagonal tile (keys [128tk,128tk+128) vs queries
        # [512j+c)), columns c < delta = 128tk-512j are fully masked — skip
        # them in S/exp/sum/PV and only apply the triangular mask to the rest.
        j, h, tk = steps[idx]
        qh = qkT[:, h * T:(h + 1) * T]
        kh = qkT[:, (2 + h) * T:(3 + h) * T]
        delta = tk * 128 - j * TQB
        lo = max(delta, 0)
        s_ps = psA.tile([128, TQB], F32, tag="a", name=f"s{j}{h}{tk}")
        nc.tensor.matmul(
            s_ps[:, lo:],
            kh[:, tk * 128:(tk + 1) * 128],
            qh[:, j * TQB + lo:(j + 1) * TQB],
            start=True, stop=True)
        p_sb = ppt.tile([128, TQB], BF, tag="p", name=f"p{j}{h}{tk}")
        nc.scalar.activation(
            p_sb[:, lo:], s_ps[:, lo:], mybir.ActivationFunctionType.Exp,
            scale=float(SCALE))
        if delta >= 0:                  # diagonal tile: causal mask
            nc.vector.tensor_mul(
                p_sb[:, lo:], p_sb[:, lo:],
                mask_sb[:, 384 - delta + lo: 896 - delta])
        p_tiles[idx] = (p_sb, lo)

    for idx in range(min(LOOKAHEAD, n_steps)):
        emit_s_exp(idx)

    sum_ps = {}
    o_ps = {}
    o_sb = {}
    for idx in range(n_steps):
        j, h, tk = steps[idx]
        if idx + LOOKAHEAD < n_steps:
            emit_s_exp(idx + LOOKAHEAD)
        n_tk = 4 * (j + 1)
        if tk == 0:
            sum_ps[j, h] = psB.tile([128, TQB], F32, tag="b", name=f"sum{j}{h}")
            o_ps[j, h] = psC.tile([128, TQB], F32, tag="c", name=f"o{j}{h}")
        p_sb, lo = p_tiles.pop(idx)
        nc.tensor.matmul(
            sum_ps[j, h][:, lo:], ones_b[:], p_sb[:, lo:],
            start=(tk == 0), stop=(tk == n_tk - 1))
        nc.tensor.matmul(
            o_ps[j, h][:, lo:],
            v_sb[:, tk * FPC + h * 128: tk * FPC + (h + 1) * 128],
            p_sb[:, lo:],
            start=(tk == 0), stop=(tk == n_tk - 1))
        if tk == n_tk - 1:
            # normalize this head's block: o_sb = o_ps / sum
            inv = smp.tile([128, TQB], F32, tag="inv", name=f"inv{j}{h}")
            nc.vector.reciprocal(inv[:], sum_ps.pop((j, h))[:])
            ot = osp.tile([128, TQB], BF, tag="osb", name=f"ot{j}{h}")
            nc.vector.tensor_mul(ot[:], o_ps.pop((j, h))[:], inv[:])
            o_sb[j, h] = ot
            if h == HPC - 1:
                emit_proj_block(nc, wpL_sb, o_sb, yp, psA, y_part, rs_out, yb,
                                j, sim_single_core)

    agp_dummy = None  # (kept name parity with v1; nothing to clean up)
    del agp_dummy


def emit_proj_block(nc, wpL_sb, o_sb, yp, psA, y_part, rs_out, yb, j,
                    sim_single_core):
    o0 = o_sb.pop((j, 0))
    o1 = o_sb.pop((j, 1))
    for t in range(ND):
        ps = psA.tile([128, TQB], F32, tag="a", name=f"proj{j}{t}")
        nc.tensor.matmul(
            ps[:], wpL_sb[:, 0 * D + t * 128: 0 * D + (t + 1) * 128], o0[:],
            start=True, stop=False)
        nc.tensor.matmul(
            ps[:], wpL_sb[:, 1 * D + t * 128: 1 * D + (t + 1) * 128], o1[:],
            start=False, stop=True)
        y_sb = yp.tile([128, TQB], BF, tag="ysb", name=f"ysb{j}{t}")
        nc.vector.tensor_copy(y_sb[:], ps[:])
        nc.sync.dma_start(y_part[j][t * 128:(t + 1) * 128, :], y_sb[:])
    if sim_single_core:
        nc.sync.dma_start(rs_out[j][:, :], y_part[j][0:FPC, :])
    else:
        nc.gpsimd.collective_compute(
            "ReduceScatter", mybir.AluOpType.add,
            replica_groups=[list(range(N_CORES))],
            ins=[y_part[j][:]], outs=[rs_out[j][:]])
    nc.sync.dma_start(yb[j][:, :], rs_out[j][:, :])


def make_mask_band() -> np.ndarray:
    p = np.arange(128)[:, None]
    j = np.arange(896)[None, :]
    return (p <= j - 384).astype(ml_dtypes.bfloat16)


def prepare_in_maps(x, W_attn, W_proj):
    x = np.asarray(x, dtype=np.float32)
    W_attn = np.asarray(W_attn, dtype=np.float32)
    W_proj = np.asarray(W_proj, dtype=np.float32)
    xT = np.ascontiguousarray(x.T).astype(ml_dtypes.bfloat16)
    mask = make_mask_band()
    in_maps = []
    for r in range(N_CORES):
        rows = slice(r * FPC, (r + 1) * FPC)
        w_qkv = np.concatenate(
            [W_attn[0 * D:][rows], W_attn[1 * D:][rows], W_attn[2 * D:][rows]],
            axis=0)                                   # [768, D]
        in_maps.append({
            "xT": xT,
            "wqkvT": np.ascontiguousarray(w_qkv.T).astype(ml_dtypes.bfloat16),
            "wpL": np.ascontiguousarray(W_proj[:, rows].T).astype(ml_dtypes.bfloat16),
            "maskB": mask,
        })
    return in_maps


def postprocess(results) -> np.ndarray:
    # core r returns y^T rows [256r, 256r+256) as NTQ column blocks
    rows = []
    for r in range(N_CORES):
        yt = np.concatenate([results[r][f"yb{j}"] for j in range(NTQ)], axis=1)
        rows.append(yt)
    yT = np.concatenate(rows, axis=0).astype(np.float32)   # [D, T]
    return np.ascontiguousarray(yT.T)


def kernel(x, W_attn, W_proj) -> np.ndarray:
    nc = build_nc()
    in_maps = prepare_in_maps(x, W_attn, W_proj)
    res = bass_utils.run_bass_kernel_spmd(
        nc, in_maps, core_ids=list(range(N_CORES)), trace=False)
    return postprocess(res.results)


# revision 19
# speedup vs baseline: 1.0500x; 1.0500x over previous
"""Causal self-attention (T=2048, D=2048, H=16) on 8 Trainium2 NeuronCores.

Head-sharded tensor parallel: 2 heads per core. Each core computes its heads'
q/k/v projections (bf16), causal attention, then projects its OWN 256
attention-output features through the matching W_proj columns (input read
straight from SBUF — no collective before the projection) producing a partial
y^T [D, T]; a per-tq-block ReduceScatter sums partials across cores and hands
core r rows [256r, 256r+256) — exactly its output slice. The host
concatenates and casts bf16 -> fp32.

Layouts (all feature/d-major so the PE contracts along partitions):
  - xT      [D, T]   bf16 : x transposed (host-side)
  - wqkvT   [D, 768] bf16 : this core's W_attn rows (q0 q1 k0 k1 v0 v1), transposed
  - wpL     [256, D] bf16 : W_proj[:, own 256 cols].T (din-major)
  - qkT     [128, 4T] bf16: q/k per head, feature on partition
  - v_sb    [tok, 256] bf16 (token on partition) so P.T@V needs no transpose
  - S_T     [tk, tq] so softmax sums run via an all-ones matmul on the PE
Attention is one flat software-pipelined stream over (j, head, tk): the S
matmul + exp for step i+LOOKAHEAD are emitted before the sum/PV matmuls of
step i, so the PE never waits on the ACT exp round trip.  The projection for
block j is emitted right after its normalize and its ReduceScatter overlaps
attention of block j+1.
"""

import numpy as np
import ml_dtypes

import concourse.bacc as bacc
import concourse.bass_utils as bass_utils
import concourse.mybir as mybir
import concourse.tile as tile

T = 2048
D = 2048
H = 16
C = 128
N_CORES = 8
HPC = H // N_CORES          # heads per core = 2
FPC = HPC * C               # features per core = 256
TQB = 512                   # tq block (PSUM free-dim limit for fp32)
NTQ = T // TQB              # 4
NKT = T // 128              # 16 tk tiles
ND = D // 128               # 16 contraction tiles
LOOKAHEAD = 3
TRAP = True                 # trapezoid (skip fully-masked diagonal columns)
PROJ_DELAY = 1              # consumer steps to defer each block's projection
ACT_COPY = "alt"            # proj PSUM->SBUF copies: True=ACT, False=DVE, "alt"=both
SCALE = 1.0 / np.sqrt(np.float32(C))

BF = mybir.dt.bfloat16
F32 = mybir.dt.float32

_NC_CACHE = {}


def build_nc(sim_single_core=False, reps=1):
    key = ("sim" if sim_single_core else "nc") + (
        f"_{reps}_{TRAP}_{LOOKAHEAD}_{PROJ_DELAY}_{ACT_COPY}")
    if key in _NC_CACHE:
        return _NC_CACHE[key]
    ndev = 1 if sim_single_core else N_CORES
    nc = bacc.Bacc("TRN2", target_bir_lowering=False, debug=False, num_devices=ndev)

    xT = nc.dram_tensor("xT", [D, T], BF, kind="ExternalInput").ap()
    wqkvT = nc.dram_tensor("wqkvT", [D, 3 * FPC], BF, kind="ExternalInput").ap()
    wpL = nc.dram_tensor("wpL", [FPC, D], BF, kind="ExternalInput").ap()
    # mask band: maskB[p, j] = 1.0 if p <= j - 384 else 0.0  (j in [0, 896))
    maskB = nc.dram_tensor("maskB", [128, 896], BF, kind="ExternalInput").ap()
    yb = [nc.dram_tensor(f"yb{j}", [FPC, TQB], BF, kind="ExternalOutput").ap()
          for j in range(NTQ)]
    y_part = [nc.dram_tensor(f"yp{j}", [D, TQB], BF, kind="Internal").ap()
              for j in range(NTQ)]
    rs_out = [nc.dram_tensor(f"rs{j}", [FPC, TQB], BF, kind="Internal").ap()
              for j in range(NTQ)]

    with tile.TileContext(nc) as tc:
        with tc.tile_pool(name="persist", bufs=1) as pp, \
             tc.tile_pool(name="ptiles", bufs=8) as ppt, \
             tc.tile_pool(name="small", bufs=2) as smp, \
             tc.tile_pool(name="osb", bufs=4) as osp, \
             tc.tile_pool(name="yout", bufs=4) as yp, \
             tc.tile_pool(name="psA", bufs=4, space="PSUM") as psA, \
             tc.tile_pool(name="psB", bufs=2, space="PSUM") as psB, \
             tc.tile_pool(name="psC", bufs=2, space="PSUM") as psC:

            mask_sb = pp.tile([128, 896], BF, tag="mask")
            ones_f = pp.tile([128, 128], F32, tag="onesf")
            ones_b = pp.tile([128, 128], BF, tag="onesb")
            nc.vector.memset(ones_f[:], 1.0)
            nc.vector.tensor_copy(ones_b[:], ones_f[:])
            nc.sync.dma_start(mask_sb[:], maskB[:])

            for _rep in range(reps):
                emit_body(nc, tc, pp, ppt, smp, osp, yp, psA, psB, psC,
                          xT, wqkvT, wpL, yb, y_part, rs_out,
                          mask_sb, ones_b, sim_single_core)

    nc.compile()
    _NC_CACHE[key] = nc
    return nc


def emit_body(nc, tc, pp, ppt, smp, osp, yp, psA, psB, psC,
              xT, wqkvT, wpL, yb, y_part, rs_out, mask_sb, ones_b,
              sim_single_core):
    # ---- phase 1: QKV projections ----
    # qkT layout: feature-block fb in {q_h0, q_h1, k_h0, k_h1} at cols
    # [fb*T, (fb+1)*T); v_sb: tok-tile tt at cols [tt*FPC, ...).
    qkT = pp.tile([128, 4 * T], BF, tag="qkT")               # 16KB/part
    v_sb = pp.tile([128, NKT * FPC], BF, tag="v")            # 8KB/part
    ph1_cm = tc.tile_pool(name="ph1", bufs=1)
    sp_cm = tc.tile_pool(name="stream", bufs=2)
    ph1 = ph1_cm.__enter__()
    sp = sp_cm.__enter__()
    w_sb = ph1.tile([128, ND * 3 * FPC], BF, tag="wbig")     # 24KB/part
    wpL_sb = pp.tile([128, HPC * D], BF, tag="wp")           # 8KB/part
    xcols = []
    for tb in range(NTQ):
        xcols.append(sp.tile([128, ND * TQB], BF, tag="xcol", name=f"xcol{tb}"))
    # interleave DMA emission so the first-needed tiles land first
    for t in range(ND):
        nc.sync.dma_start(
            w_sb[:, t * 3 * FPC:(t + 1) * 3 * FPC],
            wqkvT[t * 128:(t + 1) * 128, :])
        nc.sync.dma_start(
            xcols[0][:, t * TQB:(t + 1) * TQB],
            xT[t * 128:(t + 1) * 128, 0:TQB])
    for h in range(HPC):
        nc.sync.dma_start(
            wpL_sb[:, h * D:(h + 1) * D], wpL[h * 128:(h + 1) * 128, :])
    for tb in range(NTQ):
        xcol = xcols[tb]
        if tb > 0:
            for t in range(ND):
                nc.sync.dma_start(
                    xcol[:, t * TQB:(t + 1) * TQB],
                    xT[t * 128:(t + 1) * 128, tb * TQB:(tb + 1) * TQB])
        for fb in range(4):
            ps = psA.tile([128, TQB], F32, tag="a")
            for t in range(ND):
                nc.tensor.matmul(
                    ps[:],
                    w_sb[:, t * 3 * FPC + fb * 128: t * 3 * FPC + fb * 128 + 128],
                    xcol[:, t * TQB:(t + 1) * TQB],
                    start=(t == 0), stop=(t == ND - 1))
            nc.vector.tensor_copy(
                qkT[:, fb * T + tb * TQB: fb * T + (tb + 1) * TQB], ps[:])
        for tt in range(4):
            tok = tb * 4 + tt
            ps = psB.tile([128, FPC], F32, tag="b")
            for t in range(ND):
                nc.tensor.matmul(
                    ps[:],
                    xcol[:, t * TQB + tt * 128: t * TQB + (tt + 1) * 128],
                    w_sb[:, t * 3 * FPC + 2 * FPC:(t + 1) * 3 * FPC],
                    start=(t == 0), stop=(t == ND - 1))
            nc.vector.tensor_copy(v_sb[:, tok * FPC:(tok + 1) * FPC], ps[:])

    sp_cm.__exit__(None, None, None)
    ph1_cm.__exit__(None, None, None)

    # ---- phase 2+3: flat attention + projection stream ----
    steps = [(j, h, tk)
             for j in range(NTQ) for h in range(HPC) for tk in range(4 * (j + 1))]
    n_steps = len(steps)

    p_tiles = {}

    def emit_s_exp(idx):
        # Trapezoid: for a diagonal tile (keys [128tk,128tk+128) vs queries
        # [512j+c)), columns c < delta = 128tk-512j are fully masked — skip
        # them in S/exp/sum/PV and only apply the triangular mask to the rest.
        j, h, tk = steps[idx]
        qh = qkT[:, h * T:(h + 1) * T]
        kh = qkT[:, (2 + h) * T:(3 + h) * T]
        delta = tk * 128 - j * TQB
        lo = max(delta, 0) if TRAP else 0
        s_ps = psA.tile([128, TQB], F32, tag="a", name=f"s{j}{h}{tk}")
        nc.tensor.matmul(
            s_ps[:, lo:],
            kh[:, tk * 128:(tk + 1) * 128],
            qh[:, j * TQB + lo:(j + 1) * TQB],
            start=True, stop=True)
        p_sb = ppt.tile([128, TQB], BF, tag="p", name=f"p{j}{h}{tk}")
        nc.scalar.activation(
            p_sb[:, lo:], s_ps[:, lo:], mybir.ActivationFunctionType.Exp,
            scale=float(SCALE))
        if delta >= 0:                  # diagonal tile: causal mask
            nc.vector.tensor_mul(
                p_sb[:, lo:], p_sb[:, lo:],
                mask_sb[:, 384 - delta + lo: 896 - delta])
        p_tiles[idx] = (p_sb, lo)

    for idx in range(min(LOOKAHEAD, n_steps)):
        emit_s_exp(idx)

    sum_ps = {}
    o_ps = {}
    o_sb = {}
    proj_pending = []
    for idx in range(n_steps):
        j, h, tk = steps[idx]
        if idx + LOOKAHEAD < n_steps:
            emit_s_exp(idx + LOOKAHEAD)
        n_tk = 4 * (j + 1)
        if tk == 0:
            sum_ps[j, h] = psB.tile([128, TQB], F32, tag="b", name=f"sum{j}{h}")
            o_ps[j, h] = psC.tile([128, TQB], F32, tag="c", name=f"o{j}{h}")
        p_sb, lo = p_tiles.pop(idx)
        nc.tensor.matmul(
            sum_ps[j, h][:, lo:], ones_b[:], p_sb[:, lo:],
            start=(tk == 0), stop=(tk == n_tk - 1))
        nc.tensor.matmul(
            o_ps[j, h][:, lo:],
            v_sb[:, tk * FPC + h * 128: tk * FPC + (h + 1) * 128],
            p_sb[:, lo:],
            start=(tk == 0), stop=(tk == n_tk - 1))
        if tk == n_tk - 1:
            # normalize this head's block: o_sb = o_ps / sum
            inv = smp.tile([128, TQB], F32, tag="inv", name=f"inv{j}{h}")
            nc.vector.reciprocal(inv[:], sum_ps.pop((j, h))[:])
            ot = osp.tile([128, TQB], BF, tag="osb", name=f"ot{j}{h}")
            nc.vector.tensor_mul(ot[:], o_ps.pop((j, h))[:], inv[:])
            o_sb[j, h] = ot
            if h == HPC - 1:
                # Queue the projection: it is drained 2 tiles per consumer
                # step of the NEXT block so its PSUM tiles never monopolize
                # the psA FIFO (a 16-tile burst starves S-tile allocation,
                # which starves the exp pipeline, which stalls the PE).
                proj_pending.append([idx + PROJ_DELAY, j, 0])
        if idx == n_steps - 1:
            while proj_pending:
                item = proj_pending[0]
                emit_proj_tiles(nc, wpL_sb, o_sb, yp, psA, y_part, rs_out,
                                yb, item, ND, sim_single_core)
                if item[2] >= ND:
                    proj_pending.pop(0)
        elif proj_pending and proj_pending[0][0] <= idx:
            item = proj_pending[0]
            emit_proj_tiles(nc, wpL_sb, o_sb, yp, psA, y_part, rs_out, yb,
                            item, 2, sim_single_core)
            if item[2] >= ND:
                proj_pending.pop(0)


def emit_proj_tiles(nc, wpL_sb, o_sb, yp, psA, y_part, rs_out, yb, item,
                    budget, sim_single_core):
    _, j, _ = item
    o0 = o_sb[j, 0]
    o1 = o_sb[j, 1]
    for _ in range(budget):
        t = item[2]
        if t >= ND:
            break
        ps = psA.tile([128, TQB], F32, tag="a", name=f"proj{j}{t}")
        nc.tensor.matmul(
            ps[:], wpL_sb[:, 0 * D + t * 128: 0 * D + (t + 1) * 128], o0[:],
            start=True, stop=False)
        nc.tensor.matmul(
            ps[:], wpL_sb[:, 1 * D + t * 128: 1 * D + (t + 1) * 128], o1[:],
            start=False, stop=True)
        y_sb = yp.tile([128, TQB], BF, tag="ysb", name=f"ysb{j}{t}")
        use_act = ACT_COPY is True or (ACT_COPY == "alt" and t % 2 == 0)
        if use_act:
            nc.scalar.activation(
                y_sb[:], ps[:], mybir.ActivationFunctionType.Copy, scale=1.0)
        else:
            nc.vector.tensor_copy(y_sb[:], ps[:])
        nc.sync.dma_start(y_part[j][t * 128:(t + 1) * 128, :], y_sb[:])
        item[2] = t + 1
    if item[2] >= ND:
        del o_sb[j, 0]
        del o_sb[j, 1]
        if sim_single_core:
            nc.sync.dma_start(rs_out[j][:, :], y_part[j][0:FPC, :])
        else:
            nc.gpsimd.collective_compute(
                "ReduceScatter", mybir.AluOpType.add,
                replica_groups=[list(range(N_CORES))],
                ins=[y_part[j][:]], outs=[rs_out[j][:]])
        nc.sync.dma_start(yb[j][:, :], rs_out[j][:, :])


def make_mask_band() -> np.ndarray:
    p = np.arange(128)[:, None]
    j = np.arange(896)[None, :]
    return (p <= j - 384).astype(ml_dtypes.bfloat16)


def prepare_in_maps(x, W_attn, W_proj):
    x = np.asarray(x, dtype=np.float32)
    W_attn = np.asarray(W_attn, dtype=np.float32)
    W_proj = np.asarray(W_proj, dtype=np.float32)
    xT = np.ascontiguousarray(x.T).astype(ml_dtypes.bfloat16)
    mask = make_mask_band()
    in_maps = []
    for r in range(N_CORES):
        rows = slice(r * FPC, (r + 1) * FPC)
        w_qkv = np.concatenate(
            [W_attn[0 * D:][rows], W_attn[1 * D:][rows], W_attn[2 * D:][rows]],
            axis=0)                                   # [768, D]
        in_maps.append({
            "xT": xT,
            "wqkvT": np.ascontiguousarray(w_qkv.T).astype(ml_dtypes.bfloat16),
            "wpL": np.ascontiguousarray(W_proj[:, rows].T).astype(ml_dtypes.bfloat16),
            "maskB": mask,
        })
    return in_maps


def postprocess(results) -> np.ndarray:
    # core r returns y^T rows [256r, 256r+256) as NTQ column blocks
    rows = []
    for r in range(N_CORES):
        yt = np.concatenate([results[r][f"yb{j}"] for j in range(NTQ)], axis=1)
        rows.append(yt)
    yT = np.concatenate(rows, axis=0).astype(np.float32)   # [D, T]
    return np.ascontiguousarray(yT.T)


def kernel(x, W_attn, W_proj) -> np.ndarray:
    nc = build_nc()
    in_maps = prepare_in_maps(x, W_attn, W_proj)
    res = bass_utils.run_bass_kernel_spmd(
        nc, in_maps, core_ids=list(range(N_CORES)), trace=False)
    return postprocess(res.results)
